# revision 14
# baseline (speedup 1.0000x reference)
"""Chunked block-causal attention with statically-routed per-chunk LoRA experts,
on 8 trn2 NeuronCores.

Sharding: core = 2*b + s  (b: batch 0..3, s: head-half 0..1).
Each core computes, for its batch b and its 8 heads [8s, 8s+8):
  - qkv projection restricted to its heads' q/k/v channels (+ routed LoRA)
  - block-causal attention over the growing KV cache of its heads
  - a PARTIAL output projection over its 512 o-channels (contraction slice)
The host sums the two partial projections of each batch -> full output.

v3: attention operands in bf16 (fp8 fails the 2e-2 gate: each of q/k,
es, v in e4m3 alone contributes ~2.4e-2; all-bf16 attention sims at 2.7e-3).
  scores  S_T [keys, tok] = matmul(lhsT=kT [64,128] bf16, rhs=qT [64,512])
  exp     one ACT op per key-tile PAIR: reads a 2-bank PSUM tile
          [128,2,512] f32, writes es [128,2,512] bf16 (scale=1/8 folded),
          amortizing the ~340-cycle ACT access overhead over 1024 elems.
  PV      matmul po[0:65] += lhsT=vx [128,65] bf16 (65th col = ones ->
          softmax denominators land in po row 64), rhs=es [128,512].
  norm    oT = po[0:64] * reciprocal(po[64]): reciprocal on DVE, partition
          broadcast on Pool/GPSIMD, multiply on DVE (no ones-matmul on PE).
Projections and qkv GEMMs stay float32r (full PE rate at N=512).
LoRA scale (alpha/r = 2.0) is folded into the B factors on the host; the
disabled k-segment of the qkv LoRA simply gets no delta matmuls.

PE stream is kept saturated by a fill queue: during chunk i's attention
(where ACT exp dominates), the PE emits chunk i+1's qkv GEMMs and chunk
i-1's projection groups between score/PV pairs.
"""

import os
import sys

if "/opt/trn_rl_repo" not in sys.path:
    sys.path.insert(0, "/opt/trn_rl_repo")

from contextlib import ExitStack

import ml_dtypes
import numpy as np

import concourse.bass as bass  # noqa: F401
import concourse.mybir as mybir
import concourse.tile as tile
from concourse import bacc
from concourse.bass_utils import run_bass_kernel_spmd

F32 = mybir.dt.float32
F32R = mybir.dt.float32r
BF16 = mybir.dt.bfloat16
U16 = mybir.dt.uint16
EXP = mybir.ActivationFunctionType.Exp

B, N, C = 4, 2048, 1024
NCHUNK, CS = 4, 512
R = 16
LORA_SCALE = 2.0
HPC = 8      # heads per core
DH = 64      # head dim
NCORES = 8
NKT = N // 128  # key tiles over the full sequence (16)

_PROGRAM = None
LAST_RESULT = None  # BassKernelResults of the most recent run (for test harness)


def _build_program(reps=1):
    nc = bacc.Bacc("TRN2", target_bir_lowering=False, debug=False)

    xT_d = nc.dram_tensor("xT", [C, N], BF16, kind="ExternalInput")
    wqk_d = nc.dram_tensor("wqkT", [C, 1024], BF16, kind="ExternalInput")
    wv_d = nc.dram_tensor("wvT", [C, 512], BF16, kind="ExternalInput")
    wp_d = nc.dram_tensor("wpT", [512, 1024], BF16, kind="ExternalInput")
    aT_d = nc.dram_tensor("aT", [C, NCHUNK * R], BF16, kind="ExternalInput")
    bq_d = nc.dram_tensor("bqT", [NCHUNK, R, 512], BF16, kind="ExternalInput")
    bv_d = nc.dram_tensor("bvT", [NCHUNK, R, 512], BF16, kind="ExternalInput")
    ap_d = nc.dram_tensor("apT", [NCHUNK, 512, R], BF16, kind="ExternalInput")
    bp_d = nc.dram_tensor("bpT", [NCHUNK, R, 1024], BF16, kind="ExternalInput")
    out_d = nc.dram_tensor("out", [N, C], BF16, kind="ExternalOutput")

    with tile.TileContext(nc) as tc, ExitStack() as ctx:
        ctx.enter_context(nc.allow_low_precision(
            reason="fp8 attention (softmax-normalized); f32r GEMMs; f32 PSUM"))
        wp_pool = ctx.enter_context(tc.tile_pool(name="weights", bufs=1))
        sb = ctx.enter_context(tc.tile_pool(name="sb", bufs=2))
        ps = ctx.enter_context(tc.tile_pool(name="ps", bufs=2, space="PSUM"))

        # ---- chunk-0 activations first: compute can start before the bulk
        # of the weights arrive ----
        xc0 = []
        for ct in range(8):
            t = sb.tile([128, CS], BF16, tag="xc", bufs=12, name=f"xc0_{ct}")
            nc.sync.dma_start(t[:], xap(xT_d)[ct * 128:(ct + 1) * 128, 0:CS])
            xc0.append(t)

        # ---- persistent weights, in dependency-priority order ----
        aT = []
        for ct in range(8):
            t = wp_pool.tile([128, NCHUNK * R], BF16, tag=f"aT{ct}", name=f"aT{ct}")
            nc.sync.dma_start(t[:], xap(aT_d)[ct * 128:(ct + 1) * 128, :])
            aT.append(t)
        wqk = [wp_pool.tile([128, 1024], BF16, tag=f"wqk{ct}", name=f"wqk{ct}")
               for ct in range(8)]
        for half in range(2):
            for ct in range(8):
                nc.sync.dma_start(wqk[ct][:, half * 512:(half + 1) * 512],
                                  xap(wqk_d)[ct * 128:(ct + 1) * 128,
                                             half * 512:(half + 1) * 512])
        wv = []
        for ct in range(8):
            t = wp_pool.tile([128, 512], BF16, tag=f"wv{ct}", name=f"wv{ct}")
            nc.sync.dma_start(t[:], xap(wv_d)[ct * 128:(ct + 1) * 128, :])
            wv.append(t)
        wp = []
        for ot in range(4):
            t = wp_pool.tile([128, 1024], BF16, tag=f"wp{ot}", name=f"wp{ot}")
            nc.sync.dma_start(t[:], xap(wp_d)[ot * 128:(ot + 1) * 128, :])
            wp.append(t)
        apt = [[None] * 4 for _ in range(NCHUNK)]
        for i in range(NCHUNK):
            for ot in range(4):
                t = wp_pool.tile([128, R], BF16, tag=f"apt{i}_{ot}", name=f"apt{i}_{ot}")
                nc.sync.dma_start(t[:], xap(ap_d)[i, ot * 128:(ot + 1) * 128, :])
                apt[i][ot] = t

        # ---- persistent bf16 KV state ----
        # kT16[t]: [128ch(2 heads), kt, key]
        kT16 = [wp_pool.tile([128, NKT, 128], BF16, tag=f"kT{t}", name=f"kT{t}")
                for t in range(4)]
        # vx16[i][tt]: [128key, head, 65] bf16; col 64 = 1.0 (0x3F80) so the
        # PV accumulation also produces softmax denominators in po row 64.
        vx16 = [[wp_pool.tile([128, HPC, 65], BF16, tag=f"vx{i}_{tt}",
                              name=f"vx{i}_{tt}") for tt in range(4)]
                for i in range(NCHUNK)]
        for i in range(NCHUNK):
            for tt in range(4):
                nc.vector.memset(vx16[i][tt][:, :, 64:65].bitcast(U16), 0x3F80)

        def emit_proj(i, oT, rp_s, bp_t):
            """Per-(tt,hf) projection groups as thunks, interleaved into the
            next chunk's attention phase as PE fill work."""
            def group(tt, hf):
                p = ps.tile([128, 512], F32, tag="mm", bufs=2, name=f"pp{i}_{tt}_{hf}")
                for ot in range(4):
                    nc.tensor.matmul(p[:], oT[ot][:, tt * 128:(tt + 1) * 128],
                                     wp[ot][:, hf * 512:(hf + 1) * 512],
                                     start=(ot == 0), stop=False)
                nc.tensor.matmul(p[:], rp_s[:, tt * 128:(tt + 1) * 128],
                                 bp_t[:, hf * 512:(hf + 1) * 512],
                                 start=False, stop=True)
                os_ = sb.tile([128, 512], BF16, tag="os", bufs=2, name=f"os{i}_{tt}_{hf}")
                nc.vector.tensor_copy(os_[:], p[:])
                nc.sync.dma_start(
                    xap(out_d)[i * CS + tt * 128: i * CS + (tt + 1) * 128,
                               hf * 512:(hf + 1) * 512],
                    os_[:],
                )
            return [lambda tt=tt, hf=hf: group(tt, hf)
                    for tt in range(4) for hf in range(2)]

        def emit_qkv(i, xc):
            """Thunks for chunk i's projections to q/k/v (+ LoRA), writing
            bf16 qT/kT/vx state. Returns (thunks, qT16 tiles)."""
            state = {}

            def loads():
                state["bq"] = sb.tile([R, 512], BF16, tag="bq", bufs=2, name=f"bq{i}")
                nc.sync.dma_start(state["bq"][:], xap(bq_d)[i])
                state["bv"] = sb.tile([R, 512], BF16, tag="bv", bufs=2, name=f"bv{i}")
                nc.sync.dma_start(state["bv"][:], xap(bv_d)[i])

            def prg():
                p = ps.tile([128, CS], F32, tag="aux", bufs=2, name=f"prT{i}")
                for ct in range(8):
                    nc.tensor.matmul(p[0:R, :], aT[ct][:, i * R:(i + 1) * R], xc[ct][:],
                                     start=(ct == 0), stop=(ct == 7))
                state["r"] = sb.tile([R, CS], BF16, tag="r", bufs=2, name=f"r{i}")
                nc.vector.tensor_copy(state["r"][:], p[0:R, :])

            qT16 = [sb.tile([128, CS], BF16, tag=f"qT{t}", bufs=2, name=f"qT{i}_{t}")
                    for t in range(4)]

            def qk(ot):
                p = ps.tile([128, CS], F32, tag="mm", bufs=2, name=f"pqk{i}_{ot}")
                for ct in range(8):
                    nc.tensor.matmul(p[:], wqk[ct][:, ot * 128:(ot + 1) * 128], xc[ct][:],
                                     start=(ct == 0), stop=(ct == 7 and ot >= 4))
                if ot < 4:  # LoRA delta on q segment only (k disabled)
                    nc.tensor.matmul(p[:], state["bq"][:, ot * 128:(ot + 1) * 128],
                                     state["r"][:], start=False, stop=True)
                    nc.vector.tensor_copy(qT16[ot][:], p[:])
                else:
                    nc.vector.tensor_copy(
                        kT16[ot - 4][:, i * 4:(i + 1) * 4, :],
                        p[:].rearrange("c (kt k) -> c kt k", k=128))

            def v(tt):
                p = ps.tile([128, CS], F32, tag="mm", bufs=2, name=f"pv{i}_{tt}")
                for ct in range(8):
                    nc.tensor.matmul(p[:], xc[ct][:, tt * 128:(tt + 1) * 128], wv[ct][:],
                                     start=(ct == 0), stop=False)
                nc.tensor.matmul(p[:], state["r"][:, tt * 128:(tt + 1) * 128],
                                 state["bv"][:], start=False, stop=True)
                nc.vector.tensor_copy(
                    vx16[i][tt][:, :, 0:64],
                    p[:].rearrange("k (h d) -> k h d", d=64))

            thunks = [loads, prg]
            thunks += [lambda ot=ot: qk(ot) for ot in range(8)]
            thunks += [lambda tt=tt: v(tt) for tt in range(4)]
            return thunks, qT16

        def emit_xc(i):
            xc = []
            for ct in range(8):
                t = sb.tile([128, CS], BF16, tag="xc", bufs=12, name=f"xc{i}_{ct}")
                nc.sync.dma_start(t[:], xap(xT_d)[ct * 128:(ct + 1) * 128,
                                                  i * CS:(i + 1) * CS])
                xc.append(t)
            return xc

        sched = [(r, c) for r in range(reps) for c in range(NCHUNK)]

        # chunk 0 qkv emitted directly (nothing to overlap with yet)
        thunks0, qT16_cur = emit_qkv(0, xc0)
        for th in thunks0:
            th()

        pending = []  # PE fill work: next chunk's qkv + prev chunk's proj
        bp_t_cur = sb.tile([R, 1024], BF16, tag="bp", bufs=2, name="bp0")
        nc.sync.dma_start(bp_t_cur[:], xap(bp_d)[0])

        for si, (rep, i) in enumerate(sched):
            qT16 = qT16_cur
            bp_t = bp_t_cur

            # stage the NEXT chunk's input DMA + qkv GEMMs as fill work
            if si + 1 < len(sched):
                ni = sched[si + 1][1]
                xc_next = emit_xc(ni)
                nthunks, qT16_cur = emit_qkv(ni, xc_next)
                bp_t_cur = sb.tile([R, 1024], BF16, tag="bp", bufs=2, name=f"bp{ni}")
                nc.sync.dma_start(bp_t_cur[:], xap(bp_d)[ni])
                pending = nthunks + pending  # qkv first: gates next exp stream

            # ---- attention (per local head); fp8 DoubleRow ----
            npair = 2 * (i + 1)
            oT = [sb.tile([128, CS], BF16, tag=f"oT{t}", bufs=2, name=f"oT{i}_{t}")
                  for t in range(4)]
            niter = HPC * npair
            it = 0
            for lh in range(HPC):
                t, off = lh // 2, 64 * (lh % 2)
                po = ps.tile([128, CS], F32, tag="aux", bufs=2, name=f"po{i}_{lh}")
                for p in range(npair):
                    sp = ps.tile([128, 2, CS], F32, tag="s", bufs=2,
                                 name=f"sp{i}_{lh}_{p}")
                    for j in range(2):
                        kt = 2 * p + j
                        nc.tensor.matmul(
                            sp[:, j, :],
                            kT16[t][off:off + 64, kt, :],
                            qT16[t][off:off + 64, :],
                            start=True, stop=True)
                    es = sb.tile([128, 2, CS], BF16, tag="es", bufs=3,
                                 name=f"es{i}_{lh}_{p}")
                    nc.scalar.activation(es[:], sp[:], EXP, scale=0.125)
                    for j in range(2):
                        kt = 2 * p + j
                        nc.tensor.matmul(po[0:65, :],
                                         vx16[kt // 4][kt % 4][:, lh, :],
                                         es[:, j, :],
                                         start=(kt == 0),
                                         stop=(kt == 4 * (i + 1) - 1))
                    # fill: drain pending PE work while ACT churns exp
                    it += 1
                    quota = ((len(pending) * it) // niter
                             - (len(pending) * (it - 1)) // niter)
                    for _ in range(quota):
                        if pending:
                            pending.pop(0)()
                rc = sb.tile([1, CS], F32R, tag="rc", bufs=2, name=f"rc{i}_{lh}")
                nc.vector.reciprocal(rc[:], po[64:65, :])
                rcb = sb.tile([64, CS], F32R, tag="rcb", bufs=2, name=f"rcb{i}_{lh}")
                nc.gpsimd.partition_broadcast(rcb[:], rc[:])
                nc.vector.tensor_mul(oT[t][off:off + 64, :], po[0:64, :], rcb[:])
            while pending:
                pending.pop(0)()

            # ---- output projection LoRA reduction; GEMM groups deferred
            # into the next chunk's attention phase ----
            pr2 = ps.tile([128, CS], F32, tag="aux", bufs=2, name=f"prp{i}")
            for ot in range(4):
                nc.tensor.matmul(pr2[0:R, :], apt[i][ot][:], oT[ot][:],
                                 start=(ot == 0), stop=(ot == 3))
            rp_s = sb.tile([R, CS], BF16, tag="rp", bufs=2, name=f"rp{i}")
            nc.vector.tensor_copy(rp_s[:], pr2[0:R, :])
            pending = emit_proj(i, oT, rp_s, bp_t) + pending

        # final chunk's projection
        while pending:
            pending.pop(0)()

    nc.compile()
    return nc


def xap(t):
    return t.ap()


def _prep_core_inputs(core, x, W_qkv, lora_B_qkv, aT_all, W_proj, lora_A_proj,
                      lora_B_proj, e_idx):
    b, s = divmod(core, 2)
    hsl = slice(512 * s, 512 * s + 512)
    f32 = ml_dtypes.bfloat16
    q_rows = W_qkv[512 * s: 512 * s + 512]
    k_rows = W_qkv[1024 + 512 * s: 1024 + 512 * s + 512]
    v_rows = W_qkv[2048 + 512 * s: 2048 + 512 * s + 512]
    m = {
        "xT": np.ascontiguousarray(x[b].T, dtype=f32),
        "wqkT": np.ascontiguousarray(np.concatenate([q_rows, k_rows], 0).T, dtype=f32),
        "wvT": np.ascontiguousarray(v_rows.T, dtype=f32),
        "wpT": np.ascontiguousarray(W_proj[:, hsl].T, dtype=f32),
        "aT": np.ascontiguousarray(aT_all, dtype=f32),
        "bqT": np.ascontiguousarray(
            np.stack([(LORA_SCALE * lora_B_qkv[e][512 * s: 512 * s + 512]).T
                      for e in e_idx]), dtype=f32),
        "bvT": np.ascontiguousarray(
            np.stack([(LORA_SCALE * lora_B_qkv[e][2048 + 512 * s: 2048 + 512 * s + 512]).T
                      for e in e_idx]), dtype=f32),
        "apT": np.ascontiguousarray(
            np.stack([lora_A_proj[e][:, hsl].T for e in e_idx]), dtype=f32),
        "bpT": np.ascontiguousarray(
            np.stack([(LORA_SCALE * lora_B_proj[e]).T for e in e_idx]), dtype=f32),
    }
    return m


def kernel(x, W_qkv, lora_A_qkv, lora_B_qkv, W_proj, lora_A_proj, lora_B_proj,
           expert_indices, chunk_size):
    global _PROGRAM, LAST_RESULT
    x = np.asarray(x, dtype=np.float32)
    W_qkv = np.asarray(W_qkv, dtype=np.float32)
    lora_A_qkv = np.asarray(lora_A_qkv, dtype=np.float32)
    lora_B_qkv = np.asarray(lora_B_qkv, dtype=np.float32)
    W_proj = np.asarray(W_proj, dtype=np.float32)
    lora_A_proj = np.asarray(lora_A_proj, dtype=np.float32)
    lora_B_proj = np.asarray(lora_B_proj, dtype=np.float32)
    e_idx = [int(v) for v in np.asarray(expert_indices).reshape(-1)]
    assert int(chunk_size) == CS and x.shape == (B, N, C)

    if _PROGRAM is None:
        _PROGRAM = _build_program()
    nc = _PROGRAM

    aT_all = np.ascontiguousarray(
        np.concatenate([lora_A_qkv[e].T for e in e_idx], axis=1), dtype=np.float32)
    in_maps = [
        _prep_core_inputs(c, x, W_qkv, lora_B_qkv, aT_all, W_proj, lora_A_proj,
                          lora_B_proj, e_idx)
        for c in range(NCORES)
    ]

    trace = os.environ.get("KERNEL_TRACE") == "1"
    res = run_bass_kernel_spmd(nc, in_maps, core_ids=list(range(NCORES)), trace=trace)
    LAST_RESULT = res

    out = np.empty((B, N, C), dtype=np.float32)
    for b in range(B):
        out[b] = (res.results[2 * b]["out"].astype(np.float32)
                  + res.results[2 * b + 1]["out"].astype(np.float32))
    return out


# revision 16
# speedup vs baseline: 1.0085x; 1.0085x over previous
"""Chunked block-causal attention with statically-routed per-chunk LoRA experts,
on 8 trn2 NeuronCores.

Sharding: core = 2*b + s  (b: batch 0..3, s: head-half 0..1).
Each core computes, for its batch b and its 8 heads [8s, 8s+8):
  - qkv projection restricted to its heads' q/k/v channels (+ routed LoRA)
  - block-causal attention over the growing KV cache of its heads
  - a PARTIAL output projection over its 512 o-channels (contraction slice)
The host sums the two partial projections of each batch -> full output.

v3: attention operands in bf16 (fp8 fails the 2e-2 gate: each of q/k,
es, v in e4m3 alone contributes ~2.4e-2; all-bf16 attention sims at 2.7e-3).
  scores  S_T [keys, tok] = matmul(lhsT=kT [64,128] bf16, rhs=qT [64,512])
  exp     one ACT op per key-tile PAIR: reads a 2-bank PSUM tile
          [128,2,512] f32, writes es [128,2,512] bf16 (scale=1/8 folded),
          amortizing the ~340-cycle ACT access overhead over 1024 elems.
  PV      matmul po[0:65] += lhsT=vx [128,65] bf16 (65th col = ones ->
          softmax denominators land in po row 64), rhs=es [128,512].
  norm    oT = po[0:64] * reciprocal(po[64]): reciprocal on DVE, partition
          broadcast on Pool/GPSIMD, multiply on DVE (no ones-matmul on PE).
All GEMM operands and DMA streams (x, weights, output partials) are bf16:
same PE stream rate as float32r but half the HBM traffic and SBUF footprint
(measurably faster under fleet HBM contention). PSUM accumulation is f32
throughout; partials are upcast and summed on the host. End-to-end rel err
vs the f32 reference: 4.7e-3 (gate 2e-2).
LoRA scale (alpha/r = 2.0) is folded into the B factors on the host; the
disabled k-segment of the qkv LoRA simply gets no delta matmuls.

PE stream is kept saturated by a fill queue: during chunk i's attention
(where ACT exp dominates), the PE emits chunk i+1's qkv GEMMs and chunk
i-1's projection groups between score/PV pairs.
"""

import os
import sys

if "/opt/trn_rl_repo" not in sys.path:
    sys.path.insert(0, "/opt/trn_rl_repo")

from contextlib import ExitStack

import ml_dtypes
import numpy as np

import concourse.bass as bass  # noqa: F401
import concourse.mybir as mybir
import concourse.tile as tile
from concourse import bacc
from concourse.bass_utils import run_bass_kernel_spmd

F32 = mybir.dt.float32
F32R = mybir.dt.float32r
BF16 = mybir.dt.bfloat16
U16 = mybir.dt.uint16
EXP = mybir.ActivationFunctionType.Exp

B, N, C = 4, 2048, 1024
NCHUNK, CS = 4, 512
R = 16
LORA_SCALE = 2.0
HPC = 8      # heads per core
DH = 64      # head dim
NCORES = 8
NKT = N // 128  # key tiles over the full sequence (16)

_PROGRAM = None
LAST_RESULT = None  # BassKernelResults of the most recent run (for test harness)


def _build_program(reps=1):
    nc = bacc.Bacc("TRN2", target_bir_lowering=False, debug=False)

    xT_d = nc.dram_tensor("xT", [C, N], BF16, kind="ExternalInput")
    wqk_d = nc.dram_tensor("wqkT", [C, 1024], BF16, kind="ExternalInput")
    wv_d = nc.dram_tensor("wvT", [C, 512], BF16, kind="ExternalInput")
    wp_d = nc.dram_tensor("wpT", [512, 1024], BF16, kind="ExternalInput")
    aT_d = nc.dram_tensor("aT", [C, NCHUNK * R], BF16, kind="ExternalInput")
    bq_d = nc.dram_tensor("bqT", [NCHUNK, R, 512], BF16, kind="ExternalInput")
    bv_d = nc.dram_tensor("bvT", [NCHUNK, R, 512], BF16, kind="ExternalInput")
    ap_d = nc.dram_tensor("apT", [NCHUNK, 512, R], BF16, kind="ExternalInput")
    bp_d = nc.dram_tensor("bpT", [NCHUNK, R, 1024], BF16, kind="ExternalInput")
    out_d = nc.dram_tensor("out", [N, C], BF16, kind="ExternalOutput")

    with tile.TileContext(nc) as tc, ExitStack() as ctx:
        ctx.enter_context(nc.allow_low_precision(
            reason="bf16 operands and streams; f32 PSUM accumulation"))
        wp_pool = ctx.enter_context(tc.tile_pool(name="weights", bufs=1))
        sb = ctx.enter_context(tc.tile_pool(name="sb", bufs=2))
        ps = ctx.enter_context(tc.tile_pool(name="ps", bufs=2, space="PSUM"))

        # ---- chunk-0 activations first: compute can start before the bulk
        # of the weights arrive ----
        xc0 = []
        for ct in range(8):
            t = sb.tile([128, CS], BF16, tag="xc", bufs=12, name=f"xc0_{ct}")
            nc.sync.dma_start(t[:], xap(xT_d)[ct * 128:(ct + 1) * 128, 0:CS])
            xc0.append(t)

        # ---- persistent weights, in dependency-priority order ----
        aT = []
        for ct in range(8):
            t = wp_pool.tile([128, NCHUNK * R], BF16, tag=f"aT{ct}", name=f"aT{ct}")
            nc.sync.dma_start(t[:], xap(aT_d)[ct * 128:(ct + 1) * 128, :])
            aT.append(t)
        wqk = [wp_pool.tile([128, 1024], BF16, tag=f"wqk{ct}", name=f"wqk{ct}")
               for ct in range(8)]
        for half in range(2):
            for ct in range(8):
                nc.sync.dma_start(wqk[ct][:, half * 512:(half + 1) * 512],
                                  xap(wqk_d)[ct * 128:(ct + 1) * 128,
                                             half * 512:(half + 1) * 512])
        wv = []
        for ct in range(8):
            t = wp_pool.tile([128, 512], BF16, tag=f"wv{ct}", name=f"wv{ct}")
            nc.sync.dma_start(t[:], xap(wv_d)[ct * 128:(ct + 1) * 128, :])
            wv.append(t)
        wp = []
        for ot in range(4):
            t = wp_pool.tile([128, 1024], BF16, tag=f"wp{ot}", name=f"wp{ot}")
            nc.sync.dma_start(t[:], xap(wp_d)[ot * 128:(ot + 1) * 128, :])
            wp.append(t)
        apt = [[None] * 4 for _ in range(NCHUNK)]
        for i in range(NCHUNK):
            for ot in range(4):
                t = wp_pool.tile([128, R], BF16, tag=f"apt{i}_{ot}", name=f"apt{i}_{ot}")
                nc.sync.dma_start(t[:], xap(ap_d)[i, ot * 128:(ot + 1) * 128, :])
                apt[i][ot] = t

        # ---- persistent bf16 KV state ----
        # kT16[t]: [128ch(2 heads), kt, key]
        kT16 = [wp_pool.tile([128, NKT, 128], BF16, tag=f"kT{t}", name=f"kT{t}")
                for t in range(4)]
        # vx16[i][tt]: [128key, head, 65] bf16; col 64 = 1.0 (0x3F80) so the
        # PV accumulation also produces softmax denominators in po row 64.
        vx16 = [[wp_pool.tile([128, HPC, 65], BF16, tag=f"vx{i}_{tt}",
                              name=f"vx{i}_{tt}") for tt in range(4)]
                for i in range(NCHUNK)]
        for i in range(NCHUNK):
            for tt in range(4):
                nc.vector.memset(vx16[i][tt][:, :, 64:65].bitcast(U16), 0x3F80)

        def emit_proj(i, oT, rp_s, bp_t):
            """Per-(tt,hf) projection groups as thunks, interleaved into the
            next chunk's attention phase as PE fill work."""
            def group(tt, hf):
                p = ps.tile([128, 512], F32, tag="mm", bufs=2, name=f"pp{i}_{tt}_{hf}")
                for ot in range(4):
                    nc.tensor.matmul(p[:], oT[ot][:, tt * 128:(tt + 1) * 128],
                                     wp[ot][:, hf * 512:(hf + 1) * 512],
                                     start=(ot == 0), stop=False)
                nc.tensor.matmul(p[:], rp_s[:, tt * 128:(tt + 1) * 128],
                                 bp_t[:, hf * 512:(hf + 1) * 512],
                                 start=False, stop=True)
                os_ = sb.tile([128, 512], BF16, tag="os", bufs=2, name=f"os{i}_{tt}_{hf}")
                nc.vector.tensor_copy(os_[:], p[:])
                nc.sync.dma_start(
                    xap(out_d)[i * CS + tt * 128: i * CS + (tt + 1) * 128,
                               hf * 512:(hf + 1) * 512],
                    os_[:],
                )
            return [lambda tt=tt, hf=hf: group(tt, hf)
                    for tt in range(4) for hf in range(2)]

        def emit_qkv(i, xc):
            """Thunks for chunk i's projections to q/k/v (+ LoRA), writing
            bf16 qT/kT/vx state. Returns (thunks, qT16 tiles)."""
            state = {}

            def loads():
                state["bq"] = sb.tile([R, 512], BF16, tag="bq", bufs=2, name=f"bq{i}")
                nc.sync.dma_start(state["bq"][:], xap(bq_d)[i])
                state["bv"] = sb.tile([R, 512], BF16, tag="bv", bufs=2, name=f"bv{i}")
                nc.sync.dma_start(state["bv"][:], xap(bv_d)[i])

            def prg():
                p = ps.tile([128, CS], F32, tag="aux", bufs=2, name=f"prT{i}")
                for ct in range(8):
                    nc.tensor.matmul(p[0:R, :], aT[ct][:, i * R:(i + 1) * R], xc[ct][:],
                                     start=(ct == 0), stop=(ct == 7))
                state["r"] = sb.tile([R, CS], BF16, tag="r", bufs=2, name=f"r{i}")
                nc.vector.tensor_copy(state["r"][:], p[0:R, :])

            qT16 = [sb.tile([128, CS], BF16, tag=f"qT{t}", bufs=2, name=f"qT{i}_{t}")
                    for t in range(4)]

            def qk(ot):
                p = ps.tile([128, CS], F32, tag="mm", bufs=2, name=f"pqk{i}_{ot}")
                for ct in range(8):
                    nc.tensor.matmul(p[:], wqk[ct][:, ot * 128:(ot + 1) * 128], xc[ct][:],
                                     start=(ct == 0), stop=(ct == 7 and ot >= 4))
                if ot < 4:  # LoRA delta on q segment only (k disabled)
                    nc.tensor.matmul(p[:], state["bq"][:, ot * 128:(ot + 1) * 128],
                                     state["r"][:], start=False, stop=True)
                    nc.vector.tensor_copy(qT16[ot][:], p[:])
                else:
                    nc.vector.tensor_copy(
                        kT16[ot - 4][:, i * 4:(i + 1) * 4, :],
                        p[:].rearrange("c (kt k) -> c kt k", k=128))

            def v(tt):
                p = ps.tile([128, CS], F32, tag="mm", bufs=2, name=f"pv{i}_{tt}")
                for ct in range(8):
                    nc.tensor.matmul(p[:], xc[ct][:, tt * 128:(tt + 1) * 128], wv[ct][:],
                                     start=(ct == 0), stop=False)
                nc.tensor.matmul(p[:], state["r"][:, tt * 128:(tt + 1) * 128],
                                 state["bv"][:], start=False, stop=True)
                nc.vector.tensor_copy(
                    vx16[i][tt][:, :, 0:64],
                    p[:].rearrange("k (h d) -> k h d", d=64))

            thunks = [loads, prg]
            thunks += [lambda ot=ot: qk(ot) for ot in range(8)]
            thunks += [lambda tt=tt: v(tt) for tt in range(4)]
            return thunks, qT16

        def emit_xc(i):
            xc = []
            for ct in range(8):
                t = sb.tile([128, CS], BF16, tag="xc", bufs=12, name=f"xc{i}_{ct}")
                nc.sync.dma_start(t[:], xap(xT_d)[ct * 128:(ct + 1) * 128,
                                                  i * CS:(i + 1) * CS])
                xc.append(t)
            return xc

        sched = [(r, c) for r in range(reps) for c in range(NCHUNK)]

        # chunk 0 qkv emitted directly (nothing to overlap with yet)
        thunks0, qT16_cur = emit_qkv(0, xc0)
        for th in thunks0:
            th()

        pending = []  # PE fill work: next chunk's qkv + prev chunk's proj
        bp_t_cur = sb.tile([R, 1024], BF16, tag="bp", bufs=2, name="bp0")
        nc.sync.dma_start(bp_t_cur[:], xap(bp_d)[0])

        for si, (rep, i) in enumerate(sched):
            qT16 = qT16_cur
            bp_t = bp_t_cur

            # stage the NEXT chunk's input DMA + qkv GEMMs as fill work
            if si + 1 < len(sched):
                ni = sched[si + 1][1]
                xc_next = emit_xc(ni)
                nthunks, qT16_cur = emit_qkv(ni, xc_next)
                bp_t_cur = sb.tile([R, 1024], BF16, tag="bp", bufs=2, name=f"bp{ni}")
                nc.sync.dma_start(bp_t_cur[:], xap(bp_d)[ni])
                pending = nthunks + pending  # qkv first: gates next exp stream

            # ---- attention (per local head); fp8 DoubleRow ----
            npair = 2 * (i + 1)
            oT = [sb.tile([128, CS], BF16, tag=f"oT{t}", bufs=2, name=f"oT{i}_{t}")
                  for t in range(4)]
            niter = HPC * npair
            it = 0
            for lh in range(HPC):
                t, off = lh // 2, 64 * (lh % 2)
                po = ps.tile([128, CS], F32, tag="aux", bufs=2, name=f"po{i}_{lh}")
                for p in range(npair):
                    sp = ps.tile([128, 2, CS], F32, tag="s", bufs=2,
                                 name=f"sp{i}_{lh}_{p}")
                    for j in range(2):
                        kt = 2 * p + j
                        nc.tensor.matmul(
                            sp[:, j, :],
                            kT16[t][off:off + 64, kt, :],
                            qT16[t][off:off + 64, :],
                            start=True, stop=True)
                    es = sb.tile([128, 2, CS], BF16, tag="es", bufs=3,
                                 name=f"es{i}_{lh}_{p}")
                    nc.scalar.activation(es[:], sp[:], EXP, scale=0.125)
                    for j in range(2):
                        kt = 2 * p + j
                        nc.tensor.matmul(po[0:65, :],
                                         vx16[kt // 4][kt % 4][:, lh, :],
                                         es[:, j, :],
                                         start=(kt == 0),
                                         stop=(kt == 4 * (i + 1) - 1))
                    # fill: drain pending PE work while ACT churns exp
                    it += 1
                    quota = ((len(pending) * it) // niter
                             - (len(pending) * (it - 1)) // niter)
                    for _ in range(quota):
                        if pending:
                            pending.pop(0)()
                rc = sb.tile([1, CS], F32R, tag="rc", bufs=2, name=f"rc{i}_{lh}")
                nc.vector.reciprocal(rc[:], po[64:65, :])
                rcb = sb.tile([64, CS], F32R, tag="rcb", bufs=2, name=f"rcb{i}_{lh}")
                nc.gpsimd.partition_broadcast(rcb[:], rc[:])
                nc.vector.tensor_mul(oT[t][off:off + 64, :], po[0:64, :], rcb[:])
            while pending:
                pending.pop(0)()

            # ---- output projection LoRA reduction; GEMM groups deferred
            # into the next chunk's attention phase ----
            pr2 = ps.tile([128, CS], F32, tag="aux", bufs=2, name=f"prp{i}")
            for ot in range(4):
                nc.tensor.matmul(pr2[0:R, :], apt[i][ot][:], oT[ot][:],
                                 start=(ot == 0), stop=(ot == 3))
            rp_s = sb.tile([R, CS], BF16, tag="rp", bufs=2, name=f"rp{i}")
            nc.vector.tensor_copy(rp_s[:], pr2[0:R, :])
            pending = emit_proj(i, oT, rp_s, bp_t) + pending

        # final chunk's projection
        while pending:
            pending.pop(0)()

    nc.compile()
    return nc


def xap(t):
    return t.ap()


def _prep_core_inputs(core, x, W_qkv, lora_B_qkv, aT_all, W_proj, lora_A_proj,
                      lora_B_proj, e_idx):
    b, s = divmod(core, 2)
    hsl = slice(512 * s, 512 * s + 512)
    f32 = ml_dtypes.bfloat16
    q_rows = W_qkv[512 * s: 512 * s + 512]
    k_rows = W_qkv[1024 + 512 * s: 1024 + 512 * s + 512]
    v_rows = W_qkv[2048 + 512 * s: 2048 + 512 * s + 512]
    m = {
        "xT": np.ascontiguousarray(x[b].T, dtype=f32),
        "wqkT": np.ascontiguousarray(np.concatenate([q_rows, k_rows], 0).T, dtype=f32),
        "wvT": np.ascontiguousarray(v_rows.T, dtype=f32),
        "wpT": np.ascontiguousarray(W_proj[:, hsl].T, dtype=f32),
        "aT": np.ascontiguousarray(aT_all, dtype=f32),
        "bqT": np.ascontiguousarray(
            np.stack([(LORA_SCALE * lora_B_qkv[e][512 * s: 512 * s + 512]).T
                      for e in e_idx]), dtype=f32),
        "bvT": np.ascontiguousarray(
            np.stack([(LORA_SCALE * lora_B_qkv[e][2048 + 512 * s: 2048 + 512 * s + 512]).T
                      for e in e_idx]), dtype=f32),
        "apT": np.ascontiguousarray(
            np.stack([lora_A_proj[e][:, hsl].T for e in e_idx]), dtype=f32),
        "bpT": np.ascontiguousarray(
            np.stack([(LORA_SCALE * lora_B_proj[e]).T for e in e_idx]), dtype=f32),
    }
    return m


def kernel(x, W_qkv, lora_A_qkv, lora_B_qkv, W_proj, lora_A_proj, lora_B_proj,
           expert_indices, chunk_size):
    global _PROGRAM, LAST_RESULT
    x = np.asarray(x, dtype=np.float32)
    W_qkv = np.asarray(W_qkv, dtype=np.float32)
    lora_A_qkv = np.asarray(lora_A_qkv, dtype=np.float32)
    lora_B_qkv = np.asarray(lora_B_qkv, dtype=np.float32)
    W_proj = np.asarray(W_proj, dtype=np.float32)
    lora_A_proj = np.asarray(lora_A_proj, dtype=np.float32)
    lora_B_proj = np.asarray(lora_B_proj, dtype=np.float32)
    e_idx = [int(v) for v in np.asarray(expert_indices).reshape(-1)]
    assert int(chunk_size) == CS and x.shape == (B, N, C)

    if _PROGRAM is None:
        _PROGRAM = _build_program()
    nc = _PROGRAM

    aT_all = np.ascontiguousarray(
        np.concatenate([lora_A_qkv[e].T for e in e_idx], axis=1), dtype=np.float32)
    in_maps = [
        _prep_core_inputs(c, x, W_qkv, lora_B_qkv, aT_all, W_proj, lora_A_proj,
                          lora_B_proj, e_idx)
        for c in range(NCORES)
    ]

    trace = os.environ.get("KERNEL_TRACE") == "1"
    res = run_bass_kernel_spmd(nc, in_maps, core_ids=list(range(NCORES)), trace=trace)
    LAST_RESULT = res

    out = np.empty((B, N, C), dtype=np.float32)
    for b in range(B):
        out[b] = (res.results[2 * b]["out"].astype(np.float32)
                  + res.results[2 * b + 1]["out"].astype(np.float32))
    return out


# revision 17
# speedup vs baseline: 1.6121x; 1.5986x over previous
"""Chunked block-causal attention with statically-routed per-chunk LoRA experts,
on 8 trn2 NeuronCores.

Sharding: core = 2*b + s  (b: batch 0..3, s: head-half 0..1).
Each core computes, for its batch b and its 8 heads [8s, 8s+8):
  - qkv projection restricted to its heads' q/k/v channels (+ routed LoRA)
  - block-causal attention over the growing KV cache of its heads
  - a PARTIAL output projection over its 512 o-channels (contraction slice)
The host sums the two partial projections of each batch -> full output.

v3: attention operands in bf16 (fp8 fails the 2e-2 gate: each of q/k,
es, v in e4m3 alone contributes ~2.4e-2; all-bf16 attention sims at 2.7e-3).
  scores  S_T [keys, tok] = matmul(lhsT=kT [64,128] bf16, rhs=qT [64,512])
  exp     one ACT op per key-tile PAIR: reads a 2-bank PSUM tile
          [128,2,512] f32, writes es [128,2,512] bf16 (scale=1/8 folded),
          amortizing the ~340-cycle ACT access overhead over 1024 elems.
  PV      matmul po[0:65] += lhsT=vx [128,65] bf16 (65th col = ones ->
          softmax denominators land in po row 64), rhs=es [128,512].
  norm    oT = po[0:64] * reciprocal(po[64]): reciprocal on DVE, partition
          broadcast on Pool/GPSIMD, multiply on DVE (no ones-matmul on PE).
All GEMM operands and DMA streams (x, weights, output partials) are bf16:
same PE stream rate as float32r but half the HBM traffic and SBUF footprint
(measurably faster under fleet HBM contention). PSUM accumulation is f32
throughout; partials are upcast and summed on the host. End-to-end rel err
vs the f32 reference: 4.7e-3 (gate 2e-2).
LoRA scale (alpha/r = 2.0) is folded into the B factors on the host; the
disabled k-segment of the qkv LoRA simply gets no delta matmuls.

PE stream is kept saturated by a fill queue: during chunk i's attention
(where ACT exp dominates), the PE emits chunk i+1's qkv GEMMs and chunk
i-1's projection groups between score/PV pairs.
"""

import os
import sys

if "/opt/trn_rl_repo" not in sys.path:
    sys.path.insert(0, "/opt/trn_rl_repo")

from contextlib import ExitStack

import ml_dtypes
import numpy as np

import concourse.bass as bass  # noqa: F401
import concourse.mybir as mybir
import concourse.tile as tile
from concourse import bacc
from concourse.bass_utils import run_bass_kernel_spmd

F32 = mybir.dt.float32
F32R = mybir.dt.float32r
BF16 = mybir.dt.bfloat16
U16 = mybir.dt.uint16
EXP = mybir.ActivationFunctionType.Exp

B, N, C = 4, 2048, 1024
NCHUNK, CS = 4, 512
R = 16
LORA_SCALE = 2.0
HPC = 8      # heads per core
DH = 64      # head dim
NCORES = 8
NKT = N // 128  # key tiles over the full sequence (16)

_PROGRAM = None
LAST_RESULT = None  # BassKernelResults of the most recent run (for test harness)


def _build_program(reps=1):
    nc = bacc.Bacc("TRN2", target_bir_lowering=False, debug=False)

    xT_d = nc.dram_tensor("xT", [C, N], BF16, kind="ExternalInput")
    wqk_d = nc.dram_tensor("wqkT", [C, 1024], BF16, kind="ExternalInput")
    wv_d = nc.dram_tensor("wvT", [C, 512], BF16, kind="ExternalInput")
    wp_d = nc.dram_tensor("wpT", [512, 1024], BF16, kind="ExternalInput")
    aT_d = nc.dram_tensor("aT", [C, NCHUNK * R], BF16, kind="ExternalInput")
    bq_d = nc.dram_tensor("bqT", [NCHUNK, R, 512], BF16, kind="ExternalInput")
    bv_d = nc.dram_tensor("bvT", [NCHUNK, R, 512], BF16, kind="ExternalInput")
    ap_d = nc.dram_tensor("apT", [NCHUNK, 512, R], BF16, kind="ExternalInput")
    bp_d = nc.dram_tensor("bpT", [NCHUNK, R, 1024], BF16, kind="ExternalInput")
    out_d = nc.dram_tensor("out", [N, C], BF16, kind="ExternalOutput")

    with tile.TileContext(nc) as tc, ExitStack() as ctx:
        ctx.enter_context(nc.allow_low_precision(
            reason="bf16 operands and streams; f32 PSUM accumulation"))
        wp_pool = ctx.enter_context(tc.tile_pool(name="weights", bufs=1))
        sb = ctx.enter_context(tc.tile_pool(name="sb", bufs=2))
        ps = ctx.enter_context(tc.tile_pool(name="ps", bufs=2, space="PSUM"))

        # ---- chunk-0 activations first: compute can start before the bulk
        # of the weights arrive ----
        xc0 = []
        for ct in range(8):
            t = sb.tile([128, CS], BF16, tag="xc", bufs=12, name=f"xc0_{ct}")
            nc.sync.dma_start(t[:], xap(xT_d)[ct * 128:(ct + 1) * 128, 0:CS])
            xc0.append(t)

        # ---- persistent weights, in dependency-priority order ----
        aT = []
        for ct in range(8):
            t = wp_pool.tile([128, NCHUNK * R], BF16, tag=f"aT{ct}", name=f"aT{ct}")
            nc.sync.dma_start(t[:], xap(aT_d)[ct * 128:(ct + 1) * 128, :])
            aT.append(t)
        wqk = [wp_pool.tile([128, 1024], BF16, tag=f"wqk{ct}", name=f"wqk{ct}")
               for ct in range(8)]
        for half in range(2):
            for ct in range(8):
                nc.sync.dma_start(wqk[ct][:, half * 512:(half + 1) * 512],
                                  xap(wqk_d)[ct * 128:(ct + 1) * 128,
                                             half * 512:(half + 1) * 512])
        wv = []
        for ct in range(8):
            t = wp_pool.tile([128, 512], BF16, tag=f"wv{ct}", name=f"wv{ct}")
            nc.sync.dma_start(t[:], xap(wv_d)[ct * 128:(ct + 1) * 128, :])
            wv.append(t)
        wp = []
        for ot in range(4):
            t = wp_pool.tile([128, 1024], BF16, tag=f"wp{ot}", name=f"wp{ot}")
            nc.sync.dma_start(t[:], xap(wp_d)[ot * 128:(ot + 1) * 128, :])
            wp.append(t)
        apt = [[None] * 4 for _ in range(NCHUNK)]
        for i in range(NCHUNK):
            for ot in range(4):
                t = wp_pool.tile([128, R], BF16, tag=f"apt{i}_{ot}", name=f"apt{i}_{ot}")
                nc.sync.dma_start(t[:], xap(ap_d)[i, ot * 128:(ot + 1) * 128, :])
                apt[i][ot] = t

        # ---- persistent bf16 KV state ----
        # kT16[t]: [128ch(2 heads), kt, key]
        kT16 = [wp_pool.tile([128, NKT, 128], BF16, tag=f"kT{t}", name=f"kT{t}")
                for t in range(4)]
        # vx16[i][tt]: [128key, head, 65] bf16; col 64 = 1.0 (0x3F80) so the
        # PV accumulation also produces softmax denominators in po row 64.
        vx16 = [[wp_pool.tile([128, HPC, 65], BF16, tag=f"vx{i}_{tt}",
                              name=f"vx{i}_{tt}") for tt in range(4)]
                for i in range(NCHUNK)]
        for i in range(NCHUNK):
            for tt in range(4):
                nc.vector.memset(vx16[i][tt][:, :, 64:65].bitcast(U16), 0x3F80)

        def emit_proj(i, oT, rp_s, bp_t):
            """Per-(tt,hf) projection groups as thunks, interleaved into the
            next chunk's attention phase as PE fill work."""
            def group(tt, hf):
                p = ps.tile([128, 512], F32, tag="mm", bufs=2, name=f"pp{i}_{tt}_{hf}")
                for ot in range(4):
                    nc.tensor.matmul(p[:], oT[ot][:, tt * 128:(tt + 1) * 128],
                                     wp[ot][:, hf * 512:(hf + 1) * 512],
                                     start=(ot == 0), stop=False)
                nc.tensor.matmul(p[:], rp_s[:, tt * 128:(tt + 1) * 128],
                                 bp_t[:, hf * 512:(hf + 1) * 512],
                                 start=False, stop=True)
                os_ = sb.tile([128, 512], BF16, tag="os", bufs=2, name=f"os{i}_{tt}_{hf}")
                # balance DVE vs ACT: half the output copies go to the scalar
                # engine (Copy shares the exp activation table -> no reload)
                if (tt + hf) % 2 == 0:
                    nc.vector.tensor_copy(os_[:], p[:])
                else:
                    nc.scalar.copy(os_[:], p[:])
                nc.sync.dma_start(
                    xap(out_d)[i * CS + tt * 128: i * CS + (tt + 1) * 128,
                               hf * 512:(hf + 1) * 512],
                    os_[:],
                )
            return [lambda tt=tt, hf=hf: group(tt, hf)
                    for tt in range(4) for hf in range(2)]

        def emit_qkv(i, xc):
            """Thunks for chunk i's projections to q/k/v (+ LoRA), writing
            bf16 qT/kT/vx state. Returns (thunks, qT16 tiles)."""
            state = {}

            def loads():
                state["bq"] = sb.tile([R, 512], BF16, tag="bq", bufs=2, name=f"bq{i}")
                nc.sync.dma_start(state["bq"][:], xap(bq_d)[i])
                state["bv"] = sb.tile([R, 512], BF16, tag="bv", bufs=2, name=f"bv{i}")
                nc.sync.dma_start(state["bv"][:], xap(bv_d)[i])

            def prg():
                p = ps.tile([128, CS], F32, tag="aux", bufs=2, name=f"prT{i}")
                for ct in range(8):
                    nc.tensor.matmul(p[0:R, :], aT[ct][:, i * R:(i + 1) * R], xc[ct][:],
                                     start=(ct == 0), stop=(ct == 7))
                state["r"] = sb.tile([R, CS], BF16, tag="r", bufs=2, name=f"r{i}")
                nc.vector.tensor_copy(state["r"][:], p[0:R, :])

            qT16 = [sb.tile([128, CS], BF16, tag=f"qT{t}", bufs=2, name=f"qT{i}_{t}")
                    for t in range(4)]

            def qk(ot):
                p = ps.tile([128, CS], F32, tag="mm", bufs=2, name=f"pqk{i}_{ot}")
                for ct in range(8):
                    nc.tensor.matmul(p[:], wqk[ct][:, ot * 128:(ot + 1) * 128], xc[ct][:],
                                     start=(ct == 0), stop=(ct == 7 and ot >= 4))
                if ot < 4:  # LoRA delta on q segment only (k disabled)
                    nc.tensor.matmul(p[:], state["bq"][:, ot * 128:(ot + 1) * 128],
                                     state["r"][:], start=False, stop=True)
                    nc.vector.tensor_copy(qT16[ot][:], p[:])
                else:
                    nc.vector.tensor_copy(
                        kT16[ot - 4][:, i * 4:(i + 1) * 4, :],
                        p[:].rearrange("c (kt k) -> c kt k", k=128))

            def v(tt):
                p = ps.tile([128, CS], F32, tag="mm", bufs=2, name=f"pv{i}_{tt}")
                for ct in range(8):
                    nc.tensor.matmul(p[:], xc[ct][:, tt * 128:(tt + 1) * 128], wv[ct][:],
                                     start=(ct == 0), stop=False)
                nc.tensor.matmul(p[:], state["r"][:, tt * 128:(tt + 1) * 128],
                                 state["bv"][:], start=False, stop=True)
                nc.vector.tensor_copy(
                    vx16[i][tt][:, :, 0:64],
                    p[:].rearrange("k (h d) -> k h d", d=64))

            thunks = [loads, prg]
            thunks += [lambda ot=ot: qk(ot) for ot in range(8)]
            thunks += [lambda tt=tt: v(tt) for tt in range(4)]
            return thunks, qT16

        def emit_xc(i):
            xc = []
            for ct in range(8):
                t = sb.tile([128, CS], BF16, tag="xc", bufs=12, name=f"xc{i}_{ct}")
                nc.sync.dma_start(t[:], xap(xT_d)[ct * 128:(ct + 1) * 128,
                                                  i * CS:(i + 1) * CS])
                xc.append(t)
            return xc

        sched = [(r, c) for r in range(reps) for c in range(NCHUNK)]

        # chunk 0 qkv emitted directly (nothing to overlap with yet)
        thunks0, qT16_cur = emit_qkv(0, xc0)
        for th in thunks0:
            th()

        pending = []  # PE fill work: next chunk's qkv + prev chunk's proj
        bp_t_cur = sb.tile([R, 1024], BF16, tag="bp", bufs=2, name="bp0")
        nc.sync.dma_start(bp_t_cur[:], xap(bp_d)[0])

        for si, (rep, i) in enumerate(sched):
            qT16 = qT16_cur
            bp_t = bp_t_cur

            # stage the NEXT chunk's input DMA + qkv GEMMs as fill work
            if si + 1 < len(sched):
                ni = sched[si + 1][1]
                xc_next = emit_xc(ni)
                nthunks, qT16_cur = emit_qkv(ni, xc_next)
                bp_t_cur = sb.tile([R, 1024], BF16, tag="bp", bufs=2, name=f"bp{ni}")
                nc.sync.dma_start(bp_t_cur[:], xap(bp_d)[ni])
                pending = nthunks + pending  # qkv first: gates next exp stream

            # ---- attention (per local head); fp8 DoubleRow ----
            npair = 2 * (i + 1)
            oT = [sb.tile([128, CS], BF16, tag=f"oT{t}", bufs=2, name=f"oT{i}_{t}")
                  for t in range(4)]
            niter = HPC * npair
            it = 0
            for lh in range(HPC):
                t, off = lh // 2, 64 * (lh % 2)
                po = ps.tile([128, CS], F32, tag="aux", bufs=2, name=f"po{i}_{lh}")
                for p in range(npair):
                    sp = ps.tile([128, 2, CS], F32, tag="s", bufs=2,
                                 name=f"sp{i}_{lh}_{p}")
                    for j in range(2):
                        kt = 2 * p + j
                        nc.tensor.matmul(
                            sp[:, j, :],
                            kT16[t][off:off + 64, kt, :],
                            qT16[t][off:off + 64, :],
                            start=True, stop=True)
                    es = sb.tile([128, 2, CS], BF16, tag="es", bufs=3,
                                 name=f"es{i}_{lh}_{p}")
                    nc.scalar.activation(es[:], sp[:], EXP, scale=0.125)
                    for j in range(2):
                        kt = 2 * p + j
                        nc.tensor.matmul(po[0:65, :],
                                         vx16[kt // 4][kt % 4][:, lh, :],
                                         es[:, j, :],
                                         start=(kt == 0),
                                         stop=(kt == 4 * (i + 1) - 1))
                    # fill: drain pending PE work while ACT churns exp
                    it += 1
                    quota = ((len(pending) * it) // niter
                             - (len(pending) * (it - 1)) // niter)
                    for _ in range(quota):
                        if pending:
                            pending.pop(0)()
                rc = sb.tile([1, CS], F32R, tag="rc", bufs=2, name=f"rc{i}_{lh}")
                nc.vector.reciprocal(rc[:], po[64:65, :])
                rcb = sb.tile([64, CS], F32R, tag="rcb", bufs=2, name=f"rcb{i}_{lh}")
                nc.gpsimd.partition_broadcast(rcb[:], rc[:])
                nc.vector.tensor_mul(oT[t][off:off + 64, :], po[0:64, :], rcb[:])
            while pending:
                pending.pop(0)()

            # ---- output projection LoRA reduction; GEMM groups deferred
            # into the next chunk's attention phase ----
            pr2 = ps.tile([128, CS], F32, tag="aux", bufs=2, name=f"prp{i}")
            for ot in range(4):
                nc.tensor.matmul(pr2[0:R, :], apt[i][ot][:], oT[ot][:],
                                 start=(ot == 0), stop=(ot == 3))
            rp_s = sb.tile([R, CS], BF16, tag="rp", bufs=2, name=f"rp{i}")
            nc.vector.tensor_copy(rp_s[:], pr2[0:R, :])
            pending = emit_proj(i, oT, rp_s, bp_t) + pending

        # final chunk's projection
        while pending:
            pending.pop(0)()

    nc.compile()
    return nc


def xap(t):
    return t.ap()


def _prep_core_inputs(core, x, W_qkv, lora_B_qkv, aT_all, W_proj, lora_A_proj,
                      lora_B_proj, e_idx):
    b, s = divmod(core, 2)
    hsl = slice(512 * s, 512 * s + 512)
    f32 = ml_dtypes.bfloat16
    q_rows = W_qkv[512 * s: 512 * s + 512]
    k_rows = W_qkv[1024 + 512 * s: 1024 + 512 * s + 512]
    v_rows = W_qkv[2048 + 512 * s: 2048 + 512 * s + 512]
    m = {
        "xT": np.ascontiguousarray(x[b].T, dtype=f32),
        "wqkT": np.ascontiguousarray(np.concatenate([q_rows, k_rows], 0).T, dtype=f32),
        "wvT": np.ascontiguousarray(v_rows.T, dtype=f32),
        "wpT": np.ascontiguousarray(W_proj[:, hsl].T, dtype=f32),
        "aT": np.ascontiguousarray(aT_all, dtype=f32),
        "bqT": np.ascontiguousarray(
            np.stack([(LORA_SCALE * lora_B_qkv[e][512 * s: 512 * s + 512]).T
                      for e in e_idx]), dtype=f32),
        "bvT": np.ascontiguousarray(
            np.stack([(LORA_SCALE * lora_B_qkv[e][2048 + 512 * s: 2048 + 512 * s + 512]).T
                      for e in e_idx]), dtype=f32),
        "apT": np.ascontiguousarray(
            np.stack([lora_A_proj[e][:, hsl].T for e in e_idx]), dtype=f32),
        "bpT": np.ascontiguousarray(
            np.stack([(LORA_SCALE * lora_B_proj[e]).T for e in e_idx]), dtype=f32),
    }
    return m


def kernel(x, W_qkv, lora_A_qkv, lora_B_qkv, W_proj, lora_A_proj, lora_B_proj,
           expert_indices, chunk_size):
    global _PROGRAM, LAST_RESULT
    x = np.asarray(x, dtype=np.float32)
    W_qkv = np.asarray(W_qkv, dtype=np.float32)
    lora_A_qkv = np.asarray(lora_A_qkv, dtype=np.float32)
    lora_B_qkv = np.asarray(lora_B_qkv, dtype=np.float32)
    W_proj = np.asarray(W_proj, dtype=np.float32)
    lora_A_proj = np.asarray(lora_A_proj, dtype=np.float32)
    lora_B_proj = np.asarray(lora_B_proj, dtype=np.float32)
    e_idx = [int(v) for v in np.asarray(expert_indices).reshape(-1)]
    assert int(chunk_size) == CS and x.shape == (B, N, C)

    if _PROGRAM is None:
        _PROGRAM = _build_program()
    nc = _PROGRAM

    aT_all = np.ascontiguousarray(
        np.concatenate([lora_A_qkv[e].T for e in e_idx], axis=1), dtype=np.float32)
    in_maps = [
        _prep_core_inputs(c, x, W_qkv, lora_B_qkv, aT_all, W_proj, lora_A_proj,
                          lora_B_proj, e_idx)
        for c in range(NCORES)
    ]

    trace = os.environ.get("KERNEL_TRACE") == "1"
    res = run_bass_kernel_spmd(nc, in_maps, core_ids=list(range(NCORES)), trace=trace)
    LAST_RESULT = res

    out = np.empty((B, N, C), dtype=np.float32)
    for b in range(B):
        out[b] = (res.results[2 * b]["out"].astype(np.float32)
                  + res.results[2 * b + 1]["out"].astype(np.float32))
    return out


# revision 18
# speedup vs baseline: 2.0076x; 1.2453x over previous
"""Chunked block-causal attention with statically-routed per-chunk LoRA experts,
on 8 trn2 NeuronCores.

Sharding: core = 2*b + s  (b: batch 0..3, s: head-half 0..1).
Each core computes, for its batch b and its 8 heads [8s, 8s+8):
  - qkv projection restricted to its heads' q/k/v channels (+ routed LoRA)
  - block-causal attention over the growing KV cache of its heads
  - a PARTIAL output projection over its 512 o-channels (contraction slice)
The host sums the two partial projections of each batch -> full output.

v3: attention operands in bf16 (fp8 fails the 2e-2 gate: each of q/k,
es, v in e4m3 alone contributes ~2.4e-2; all-bf16 attention sims at 2.7e-3).
  scores  S_T [keys, tok] = matmul(lhsT=kT [64,128] bf16, rhs=qT [64,512])
  exp     one ACT op per key-tile PAIR: reads a 2-bank PSUM tile
          [128,2,512] f32, writes es [128,2,512] bf16 (scale=1/8 folded),
          amortizing the ~340-cycle ACT access overhead over 1024 elems.
  PV      matmul po[0:65] += lhsT=vx [128,65] bf16 (65th col = ones ->
          softmax denominators land in po row 64), rhs=es [128,512].
  norm    oT = po[0:64] * reciprocal(po[64]): reciprocal on DVE, partition
          broadcast on Pool/GPSIMD, multiply on DVE (no ones-matmul on PE).
All GEMM operands and DMA streams (x, weights, output partials) are bf16:
same PE stream rate as float32r but half the HBM traffic and SBUF footprint
(measurably faster under fleet HBM contention). PSUM accumulation is f32
throughout; partials are upcast and summed on the host. End-to-end rel err
vs the f32 reference: 4.7e-3 (gate 2e-2).
LoRA scale (alpha/r = 2.0) is folded into the B factors on the host; the
disabled k-segment of the qkv LoRA simply gets no delta matmuls.

PE stream is kept saturated by a fill queue: during chunk i's attention
(where ACT exp dominates), the PE emits chunk i+1's qkv GEMMs and chunk
i-1's projection groups between score/PV pairs.
"""

import os
import sys

if "/opt/trn_rl_repo" not in sys.path:
    sys.path.insert(0, "/opt/trn_rl_repo")

from contextlib import ExitStack

import ml_dtypes
import numpy as np

import concourse.bass as bass  # noqa: F401
import concourse.mybir as mybir
import concourse.tile as tile
from concourse import bacc
from concourse.bass_utils import run_bass_kernel_spmd

F32 = mybir.dt.float32
F32R = mybir.dt.float32r
BF16 = mybir.dt.bfloat16
U16 = mybir.dt.uint16
EXP = mybir.ActivationFunctionType.Exp

B, N, C = 4, 2048, 1024
NCHUNK, CS = 4, 512
R = 16
LORA_SCALE = 2.0
HPC = 8      # heads per core
DH = 64      # head dim
NCORES = 8
NKT = N // 128  # key tiles over the full sequence (16)

_PROGRAM = None
LAST_RESULT = None  # BassKernelResults of the most recent run (for test harness)


def _build_program(reps=1):
    nc = bacc.Bacc("TRN2", target_bir_lowering=False, debug=False)

    xT_d = nc.dram_tensor("xT", [C, N], BF16, kind="ExternalInput")
    wqk_d = nc.dram_tensor("wqkT", [C, 1024], BF16, kind="ExternalInput")
    wv_d = nc.dram_tensor("wvT", [C, 512], BF16, kind="ExternalInput")
    wp_d = nc.dram_tensor("wpT", [512, 1024], BF16, kind="ExternalInput")
    aT_d = nc.dram_tensor("aT", [C, NCHUNK * R], BF16, kind="ExternalInput")
    bq_d = nc.dram_tensor("bqT", [NCHUNK, R, 512], BF16, kind="ExternalInput")
    bv_d = nc.dram_tensor("bvT", [NCHUNK, R, 512], BF16, kind="ExternalInput")
    ap_d = nc.dram_tensor("apT", [NCHUNK, 512, R], BF16, kind="ExternalInput")
    bp_d = nc.dram_tensor("bpT", [NCHUNK, R, 1024], BF16, kind="ExternalInput")
    out_d = nc.dram_tensor("out", [N, C], BF16, kind="ExternalOutput")

    with tile.TileContext(nc) as tc, ExitStack() as ctx:
        ctx.enter_context(nc.allow_low_precision(
            reason="bf16 operands and streams; f32 PSUM accumulation"))
        wp_pool = ctx.enter_context(tc.tile_pool(name="weights", bufs=1))
        sb = ctx.enter_context(tc.tile_pool(name="sb", bufs=2))
        ps = ctx.enter_context(tc.tile_pool(name="ps", bufs=2, space="PSUM"))

        # ---- chunk-0 activations first: compute can start before the bulk
        # of the weights arrive ----
        xc0 = []
        for ct in range(8):
            t = sb.tile([128, CS], BF16, tag="xc", bufs=12, name=f"xc0_{ct}")
            nc.sync.dma_start(t[:], xap(xT_d)[ct * 128:(ct + 1) * 128, 0:CS])
            xc0.append(t)

        # ---- persistent weights, in dependency-priority order ----
        aT = []
        for ct in range(8):
            t = wp_pool.tile([128, NCHUNK * R], BF16, tag=f"aT{ct}", name=f"aT{ct}")
            nc.sync.dma_start(t[:], xap(aT_d)[ct * 128:(ct + 1) * 128, :])
            aT.append(t)
        wqk = [wp_pool.tile([128, 1024], BF16, tag=f"wqk{ct}", name=f"wqk{ct}")
               for ct in range(8)]
        for half in range(2):
            for ct in range(8):
                nc.sync.dma_start(wqk[ct][:, half * 512:(half + 1) * 512],
                                  xap(wqk_d)[ct * 128:(ct + 1) * 128,
                                             half * 512:(half + 1) * 512])
        wv = []
        for ct in range(8):
            t = wp_pool.tile([128, 512], BF16, tag=f"wv{ct}", name=f"wv{ct}")
            nc.sync.dma_start(t[:], xap(wv_d)[ct * 128:(ct + 1) * 128, :])
            wv.append(t)
        wp = []
        for ot in range(4):
            t = wp_pool.tile([128, 1024], BF16, tag=f"wp{ot}", name=f"wp{ot}")
            nc.sync.dma_start(t[:], xap(wp_d)[ot * 128:(ot + 1) * 128, :])
            wp.append(t)
        apt = [[None] * 4 for _ in range(NCHUNK)]
        for i in range(NCHUNK):
            for ot in range(4):
                t = wp_pool.tile([128, R], BF16, tag=f"apt{i}_{ot}", name=f"apt{i}_{ot}")
                nc.sync.dma_start(t[:], xap(ap_d)[i, ot * 128:(ot + 1) * 128, :])
                apt[i][ot] = t

        # ---- persistent bf16 KV state ----
        # kT16[t]: [128ch(2 heads), kt, key]
        kT16 = [wp_pool.tile([128, NKT, 128], BF16, tag=f"kT{t}", name=f"kT{t}")
                for t in range(4)]
        # vx16[i][tt]: [128key, head, 65] bf16; col 64 = 1.0 (0x3F80) so the
        # PV accumulation also produces softmax denominators in po row 64.
        vx16 = [[wp_pool.tile([128, HPC, 65], BF16, tag=f"vx{i}_{tt}",
                              name=f"vx{i}_{tt}") for tt in range(4)]
                for i in range(NCHUNK)]
        for i in range(NCHUNK):
            for tt in range(4):
                nc.vector.memset(vx16[i][tt][:, :, 64:65].bitcast(U16), 0x3F80)

        def emit_proj(i, oT, rp_s, bp_t):
            """Per-(tt,hf) projection groups as thunks, interleaved into the
            next chunk's attention phase as PE fill work."""
            def group(tt, hf):
                p = ps.tile([128, 512], F32, tag="mm", bufs=2, name=f"pp{i}_{tt}_{hf}")
                for ot in range(4):
                    nc.tensor.matmul(p[:], oT[ot][:, tt * 128:(tt + 1) * 128],
                                     wp[ot][:, hf * 512:(hf + 1) * 512],
                                     start=(ot == 0), stop=False)
                nc.tensor.matmul(p[:], rp_s[:, tt * 128:(tt + 1) * 128],
                                 bp_t[:, hf * 512:(hf + 1) * 512],
                                 start=False, stop=True)
                os_ = sb.tile([128, 512], BF16, tag="os", bufs=2, name=f"os{i}_{tt}_{hf}")
                # offload output copies to the scalar engine: DVE is the
                # co-critical engine in quiet windows and these copies are
                # not latency-sensitive (Copy shares the exp activation
                # table -> no reload)
                nc.scalar.copy(os_[:], p[:])
                nc.sync.dma_start(
                    xap(out_d)[i * CS + tt * 128: i * CS + (tt + 1) * 128,
                               hf * 512:(hf + 1) * 512],
                    os_[:],
                )
            return [lambda tt=tt, hf=hf: group(tt, hf)
                    for tt in range(4) for hf in range(2)]

        def emit_qkv(i, xc):
            """Thunks for chunk i's projections to q/k/v (+ LoRA), writing
            bf16 qT/kT/vx state. Returns (thunks, qT16 tiles)."""
            state = {}

            def loads():
                state["bq"] = sb.tile([R, 512], BF16, tag="bq", bufs=2, name=f"bq{i}")
                nc.sync.dma_start(state["bq"][:], xap(bq_d)[i])
                state["bv"] = sb.tile([R, 512], BF16, tag="bv", bufs=2, name=f"bv{i}")
                nc.sync.dma_start(state["bv"][:], xap(bv_d)[i])

            def prg():
                p = ps.tile([128, CS], F32, tag="aux", bufs=2, name=f"prT{i}")
                for ct in range(8):
                    nc.tensor.matmul(p[0:R, :], aT[ct][:, i * R:(i + 1) * R], xc[ct][:],
                                     start=(ct == 0), stop=(ct == 7))
                state["r"] = sb.tile([R, CS], BF16, tag="r", bufs=2, name=f"r{i}")
                nc.vector.tensor_copy(state["r"][:], p[0:R, :])

            qT16 = [sb.tile([128, CS], BF16, tag=f"qT{t}", bufs=2, name=f"qT{i}_{t}")
                    for t in range(4)]

            def qk(ot):
                p = ps.tile([128, CS], F32, tag="mm", bufs=2, name=f"pqk{i}_{ot}")
                for ct in range(8):
                    nc.tensor.matmul(p[:], wqk[ct][:, ot * 128:(ot + 1) * 128], xc[ct][:],
                                     start=(ct == 0), stop=(ct == 7 and ot >= 4))
                if ot < 4:  # LoRA delta on q segment only (k disabled)
                    nc.tensor.matmul(p[:], state["bq"][:, ot * 128:(ot + 1) * 128],
                                     state["r"][:], start=False, stop=True)
                    nc.vector.tensor_copy(qT16[ot][:], p[:])
                else:
                    nc.vector.tensor_copy(
                        kT16[ot - 4][:, i * 4:(i + 1) * 4, :],
                        p[:].rearrange("c (kt k) -> c kt k", k=128))

            def v(tt):
                p = ps.tile([128, CS], F32, tag="mm", bufs=2, name=f"pv{i}_{tt}")
                for ct in range(8):
                    nc.tensor.matmul(p[:], xc[ct][:, tt * 128:(tt + 1) * 128], wv[ct][:],
                                     start=(ct == 0), stop=False)
                nc.tensor.matmul(p[:], state["r"][:, tt * 128:(tt + 1) * 128],
                                 state["bv"][:], start=False, stop=True)
                nc.vector.tensor_copy(
                    vx16[i][tt][:, :, 0:64],
                    p[:].rearrange("k (h d) -> k h d", d=64))

            thunks = [loads, prg]
            thunks += [lambda ot=ot: qk(ot) for ot in range(8)]
            thunks += [lambda tt=tt: v(tt) for tt in range(4)]
            return thunks, qT16

        def emit_xc(i):
            xc = []
            for ct in range(8):
                t = sb.tile([128, CS], BF16, tag="xc", bufs=12, name=f"xc{i}_{ct}")
                nc.sync.dma_start(t[:], xap(xT_d)[ct * 128:(ct + 1) * 128,
                                                  i * CS:(i + 1) * CS])
                xc.append(t)
            return xc

        sched = [(r, c) for r in range(reps) for c in range(NCHUNK)]

        # chunk 0 qkv emitted directly (nothing to overlap with yet)
        thunks0, qT16_cur = emit_qkv(0, xc0)
        for th in thunks0:
            th()

        pending = []  # PE fill work: next chunk's qkv + prev chunk's proj
        bp_t_cur = sb.tile([R, 1024], BF16, tag="bp", bufs=2, name="bp0")
        nc.sync.dma_start(bp_t_cur[:], xap(bp_d)[0])

        for si, (rep, i) in enumerate(sched):
            qT16 = qT16_cur
            bp_t = bp_t_cur

            # stage the NEXT chunk's input DMA + qkv GEMMs as fill work
            if si + 1 < len(sched):
                ni = sched[si + 1][1]
                xc_next = emit_xc(ni)
                nthunks, qT16_cur = emit_qkv(ni, xc_next)
                bp_t_cur = sb.tile([R, 1024], BF16, tag="bp", bufs=2, name=f"bp{ni}")
                nc.sync.dma_start(bp_t_cur[:], xap(bp_d)[ni])
                pending = nthunks + pending  # qkv first: gates next exp stream

            # ---- attention (per local head); fp8 DoubleRow ----
            npair = 2 * (i + 1)
            oT = [sb.tile([128, CS], BF16, tag=f"oT{t}", bufs=2, name=f"oT{i}_{t}")
                  for t in range(4)]
            niter = HPC * npair
            it = 0
            for lh in range(HPC):
                t, off = lh // 2, 64 * (lh % 2)
                po = ps.tile([128, CS], F32, tag="aux", bufs=2, name=f"po{i}_{lh}")
                for p in range(npair):
                    sp = ps.tile([128, 2, CS], F32, tag="s", bufs=2,
                                 name=f"sp{i}_{lh}_{p}")
                    for j in range(2):
                        kt = 2 * p + j
                        nc.tensor.matmul(
                            sp[:, j, :],
                            kT16[t][off:off + 64, kt, :],
                            qT16[t][off:off + 64, :],
                            start=True, stop=True)
                    es = sb.tile([128, 2, CS], BF16, tag="es", bufs=3,
                                 name=f"es{i}_{lh}_{p}")
                    nc.scalar.activation(es[:], sp[:], EXP, scale=0.125)
                    for j in range(2):
                        kt = 2 * p + j
                        nc.tensor.matmul(po[0:65, :],
                                         vx16[kt // 4][kt % 4][:, lh, :],
                                         es[:, j, :],
                                         start=(kt == 0),
                                         stop=(kt == 4 * (i + 1) - 1))
                    # fill: drain pending PE work while ACT churns exp
                    it += 1
                    quota = ((len(pending) * it) // niter
                             - (len(pending) * (it - 1)) // niter)
                    for _ in range(quota):
                        if pending:
                            pending.pop(0)()
                rc = sb.tile([1, CS], F32R, tag="rc", bufs=2, name=f"rc{i}_{lh}")
                nc.vector.reciprocal(rc[:], po[64:65, :])
                rcb = sb.tile([64, CS], F32R, tag="rcb", bufs=2, name=f"rcb{i}_{lh}")
                nc.gpsimd.partition_broadcast(rcb[:], rc[:])
                nc.vector.tensor_mul(oT[t][off:off + 64, :], po[0:64, :], rcb[:])
            while pending:
                pending.pop(0)()

            # ---- output projection LoRA reduction; GEMM groups deferred
            # into the next chunk's attention phase ----
            pr2 = ps.tile([128, CS], F32, tag="aux", bufs=2, name=f"prp{i}")
            for ot in range(4):
                nc.tensor.matmul(pr2[0:R, :], apt[i][ot][:], oT[ot][:],
                                 start=(ot == 0), stop=(ot == 3))
            rp_s = sb.tile([R, CS], BF16, tag="rp", bufs=2, name=f"rp{i}")
            nc.vector.tensor_copy(rp_s[:], pr2[0:R, :])
            pending = emit_proj(i, oT, rp_s, bp_t) + pending

        # final chunk's projection
        while pending:
            pending.pop(0)()

    nc.compile()
    return nc


def xap(t):
    return t.ap()


def _prep_core_inputs(core, x, W_qkv, lora_B_qkv, aT_all, W_proj, lora_A_proj,
                      lora_B_proj, e_idx):
    b, s = divmod(core, 2)
    hsl = slice(512 * s, 512 * s + 512)
    f32 = ml_dtypes.bfloat16
    q_rows = W_qkv[512 * s: 512 * s + 512]
    k_rows = W_qkv[1024 + 512 * s: 1024 + 512 * s + 512]
    v_rows = W_qkv[2048 + 512 * s: 2048 + 512 * s + 512]
    m = {
        "xT": np.ascontiguousarray(x[b].T, dtype=f32),
        "wqkT": np.ascontiguousarray(np.concatenate([q_rows, k_rows], 0).T, dtype=f32),
        "wvT": np.ascontiguousarray(v_rows.T, dtype=f32),
        "wpT": np.ascontiguousarray(W_proj[:, hsl].T, dtype=f32),
        "aT": np.ascontiguousarray(aT_all, dtype=f32),
        "bqT": np.ascontiguousarray(
            np.stack([(LORA_SCALE * lora_B_qkv[e][512 * s: 512 * s + 512]).T
                      for e in e_idx]), dtype=f32),
        "bvT": np.ascontiguousarray(
            np.stack([(LORA_SCALE * lora_B_qkv[e][2048 + 512 * s: 2048 + 512 * s + 512]).T
                      for e in e_idx]), dtype=f32),
        "apT": np.ascontiguousarray(
            np.stack([lora_A_proj[e][:, hsl].T for e in e_idx]), dtype=f32),
        "bpT": np.ascontiguousarray(
            np.stack([(LORA_SCALE * lora_B_proj[e]).T for e in e_idx]), dtype=f32),
    }
    return m


def kernel(x, W_qkv, lora_A_qkv, lora_B_qkv, W_proj, lora_A_proj, lora_B_proj,
           expert_indices, chunk_size):
    global _PROGRAM, LAST_RESULT
    x = np.asarray(x, dtype=np.float32)
    W_qkv = np.asarray(W_qkv, dtype=np.float32)
    lora_A_qkv = np.asarray(lora_A_qkv, dtype=np.float32)
    lora_B_qkv = np.asarray(lora_B_qkv, dtype=np.float32)
    W_proj = np.asarray(W_proj, dtype=np.float32)
    lora_A_proj = np.asarray(lora_A_proj, dtype=np.float32)
    lora_B_proj = np.asarray(lora_B_proj, dtype=np.float32)
    e_idx = [int(v) for v in np.asarray(expert_indices).reshape(-1)]
    assert int(chunk_size) == CS and x.shape == (B, N, C)

    if _PROGRAM is None:
        _PROGRAM = _build_program()
    nc = _PROGRAM

    aT_all = np.ascontiguousarray(
        np.concatenate([lora_A_qkv[e].T for e in e_idx], axis=1), dtype=np.float32)
    in_maps = [
        _prep_core_inputs(c, x, W_qkv, lora_B_qkv, aT_all, W_proj, lora_A_proj,
                          lora_B_proj, e_idx)
        for c in range(NCORES)
    ]

    trace = os.environ.get("KERNEL_TRACE") == "1"
    res = run_bass_kernel_spmd(nc, in_maps, core_ids=list(range(NCORES)), trace=trace)
    LAST_RESULT = res

    out = np.empty((B, N, C), dtype=np.float32)
    for b in range(B):
        out[b] = (res.results[2 * b]["out"].astype(np.float32)
                  + res.results[2 * b + 1]["out"].astype(np.float32))
    return out
